# revision 1
# baseline (speedup 1.0000x reference)
"""GCN layer on 8 TRN2 NeuronCores.

Computation (matches the reference):
    support  = x @ weight                          # [N, F]
    A        = scatter(adj, edge_w) + I            # dense [N, N], duplicate edges sum
    deg      = A.sum(axis=1)
    dis      = 1/sqrt(deg + 1e-10)
    out      = (dis[:,None] * A * dis[None,:]) @ support + bias

Strategy: fold the degree normalization into the dense adjacency on the host
(cheap O(E)/O(N) index work), materialize A'^T = (dis_r * w * dis_c) scattered
at [c, r] in bf16, and row-shard the propagation across 8 cores (1024 output
rows each).  Each core:
  - computes the full support S = x @ W on-device (x^T replicated, bf16),
  - streams its 8192x1024 bf16 A'^T shard from HBM as the matmul moving
    operand, accumulating out^T = S^T @ A'^T in PSUM over 64 k-tiles,
  - adds bias and writes its out^T chunk [128, 1024] f32.
The host transposes/concatenates the 8 chunks into the [8192, 128] output.
"""

import numpy as np
import ml_dtypes

N = 8192
F = 128
NCORES = 8
RPC = N // NCORES  # 1024 rows per core
JT = N // 128  # 64 contraction tiles
EPS = 1e-10

_graph_cache = {}


def _build_graph():
    import concourse.tile as tile
    from concourse import bacc, mybir
    from concourse.bass import ts

    nc = bacc.Bacc("TRN2", target_bir_lowering=False, debug=False, num_devices=NCORES)
    at = nc.declare_dram_parameter("at", [N, RPC], mybir.dt.bfloat16, isOutput=False)
    xt = nc.declare_dram_parameter("xt", [F, N], mybir.dt.bfloat16, isOutput=False)
    w = nc.declare_dram_parameter("w", [F, F], mybir.dt.bfloat16, isOutput=False)
    bias = nc.declare_dram_parameter("bias", [F, 1], mybir.dt.float32, isOutput=False)
    out = nc.declare_dram_parameter("out", [F, RPC], mybir.dt.float32, isOutput=True)

    with tile.TileContext(nc) as tc:
        with (
            tc.tile_pool(name="singles", bufs=1) as singles,
            tc.tile_pool(name="atp", bufs=16) as atp,
            tc.tile_pool(name="ps_s", bufs=4, space="PSUM") as ps_s,
            tc.tile_pool(name="ps_o", bufs=1, space="PSUM") as ps_o,
        ):
            xt_sb = singles.tile([F, N], mybir.dt.bfloat16)
            for i in range(8):
                nc.sync.dma_start(xt_sb[:, ts(i, N // 8)], xt[:, ts(i, N // 8)])
            w_sb = singles.tile([F, F], mybir.dt.bfloat16)
            nc.sync.dma_start(w_sb[:], w[:])
            bias_sb = singles.tile([F, 1], mybir.dt.float32)
            nc.sync.dma_start(bias_sb[:], bias[:])

            # Support S[j, f] = x @ W, in bf16, partitioned as 64 [128, 128]
            # column blocks of s_sb (block jt holds rows jt*128..jt*128+127).
            s_sb = singles.tile([F, N], mybir.dt.bfloat16)
            for jt in range(JT):
                ps = ps_s.tile([F, F], mybir.dt.float32)
                nc.tensor.matmul(
                    ps[:], xt_sb[:, ts(jt, F)], w_sb[:], start=True, stop=True
                )
                nc.vector.tensor_copy(s_sb[:, ts(jt, F)], ps[:])

            # out^T [f, i] = sum_j S[j, f] * A'T[j, i], accumulated over the
            # 64 j-tiles into two PSUM banks (i split 0:512 / 512:1024).
            po0 = ps_o.tile([F, 512], mybir.dt.float32)
            po1 = ps_o.tile([F, 512], mybir.dt.float32)
            for jt in range(JT):
                a_t = atp.tile([F, RPC], mybir.dt.bfloat16)
                nc.sync.dma_start(a_t[:], at[ts(jt, F), :])
                st = s_sb[:, ts(jt, F)]
                first, last = jt == 0, jt == JT - 1
                nc.tensor.matmul(po0[:], st, a_t[:, 0:512], start=first, stop=last)
                nc.tensor.matmul(po1[:], st, a_t[:, 512:1024], start=first, stop=last)

            out_sb = singles.tile([F, RPC], mybir.dt.float32)
            nc.vector.tensor_scalar_add(out_sb[:, 0:512], po0[:], bias_sb[:])
            nc.vector.tensor_scalar_add(out_sb[:, 512:1024], po1[:], bias_sb[:])
            nc.sync.dma_start(out[:], out_sb[:])

    nc.compile()
    return nc


def _get_graph():
    if "nc" not in _graph_cache:
        _graph_cache["nc"] = _build_graph()
    return _graph_cache["nc"]


def _prepare_in_maps(x, adj, edge_w, weight, bias):
    x = np.asarray(x, dtype=np.float32)
    adj = np.asarray(adj).astype(np.int64)
    edge_w = np.asarray(edge_w, dtype=np.float32)
    weight = np.asarray(weight, dtype=np.float32)
    bias = np.asarray(bias, dtype=np.float32)

    rows, cols = adj[0], adj[1]
    deg = 1.0 + np.bincount(rows, weights=edge_w.astype(np.float64), minlength=N)
    dis = (1.0 / np.sqrt(deg + EPS)).astype(np.float32)

    # A'^T[c, r] = dis[r] * w_e * dis[c]; diagonal gets dis[i]^2 (self loop).
    vals = edge_w * dis[rows] * dis[cols]
    at = np.zeros((N, N), dtype=np.float32)
    np.add.at(at, (cols, rows), vals)
    idx = np.arange(N)
    at[idx, idx] += dis * dis
    atb = at.astype(ml_dtypes.bfloat16)

    xtb = np.ascontiguousarray(x.T).astype(ml_dtypes.bfloat16)
    wb = weight.astype(ml_dtypes.bfloat16)
    bias_col = np.ascontiguousarray(bias.reshape(F, 1))

    return [
        {
            "at": np.ascontiguousarray(atb[:, c * RPC : (c + 1) * RPC]),
            "xt": xtb,
            "w": wb,
            "bias": bias_col,
        }
        for c in range(NCORES)
    ]


def _run(in_maps, trace=False, tmpdir=None):
    from concourse.bass_utils import run_bass_kernel_spmd

    nc = _get_graph()
    return run_bass_kernel_spmd(
        nc, in_maps, core_ids=list(range(NCORES)), trace=trace, tmpdir=tmpdir
    )


def _assemble(results):
    return np.ascontiguousarray(
        np.concatenate([results[c]["out"].T for c in range(NCORES)], axis=0)
    ).astype(np.float32)


def kernel(x, adj, edge_w, weight, bias):
    in_maps = _prepare_in_maps(x, adj, edge_w, weight, bias)
    res = _run(in_maps, trace=False)
    return _assemble(res.results)


def kernel_traced(x, adj, edge_w, weight, bias, tmpdir=None):
    """Same as kernel() but profiles the NEFF; returns (output, BassKernelResults)."""
    in_maps = _prepare_in_maps(x, adj, edge_w, weight, bias)
    res = _run(in_maps, trace=True, tmpdir=tmpdir)
    return _assemble(res.results), res


# revision 2
# speedup vs baseline: 1.0327x; 1.0327x over previous
"""GCN layer on 8 TRN2 NeuronCores.

Computation (matches the reference):
    support  = x @ weight                          # [N, F]
    A        = scatter(adj, edge_w) + I            # dense [N, N], duplicate edges sum
    deg      = A.sum(axis=1)
    dis      = 1/sqrt(deg + 1e-10)
    out      = (dis[:,None] * A * dis[None,:]) @ support + bias

Strategy: fold the degree normalization into the dense adjacency on the host
(cheap O(E)/O(N) index work), materialize A'^T = (dis_r * w * dis_c) scattered
at [c, r] in bf16, and row-shard the propagation across 8 cores (1024 output
rows each).  Each core:
  - computes the full support S = x @ W on-device (x^T replicated, bf16),
  - streams its 8192x1024 bf16 A'^T shard from HBM as the matmul moving
    operand, accumulating out^T = S^T @ A'^T in PSUM over 64 k-tiles,
  - adds bias and writes its out^T chunk [128, 1024] f32.
The host transposes/concatenates the 8 chunks into the [8192, 128] output.
"""

import numpy as np
import ml_dtypes

N = 8192
F = 128
NCORES = 8
RPC = N // NCORES  # 1024 rows per core
JT = N // 128  # 64 contraction tiles
EPS = 1e-10

_graph_cache = {}


def _build_graph():
    import concourse.tile as tile
    from concourse import bacc, mybir
    from concourse.bass import ts

    nc = bacc.Bacc("TRN2", target_bir_lowering=False, debug=False, num_devices=NCORES)
    at = nc.declare_dram_parameter("at", [N, RPC], mybir.dt.bfloat16, isOutput=False)
    xt = nc.declare_dram_parameter("xt", [F, N], mybir.dt.bfloat16, isOutput=False)
    w = nc.declare_dram_parameter("w", [F, F], mybir.dt.bfloat16, isOutput=False)
    bias = nc.declare_dram_parameter("bias", [F, 1], mybir.dt.float32, isOutput=False)
    out = nc.declare_dram_parameter("out", [F, RPC], mybir.dt.float32, isOutput=True)

    TPC = 2  # j-tiles per DMA chunk
    NCH = JT // TPC  # 32 chunks
    with tile.TileContext(nc) as tc:
        with (
            tc.tile_pool(name="singles", bufs=1) as singles,
            tc.tile_pool(name="atp", bufs=6) as atp,
            tc.tile_pool(name="ps_s", bufs=4, space="PSUM") as ps_s,
            tc.tile_pool(name="ps_o", bufs=1, space="PSUM") as ps_o,
        ):
            # Weights/x^T on the scalar queue so they don't serialize behind
            # the adjacency stream (each dma_start costs ~0.7us of issue time
            # on its sequencer).
            w_sb = singles.tile([F, F], mybir.dt.bfloat16)
            nc.scalar.dma_start(w_sb[:], w[:])
            xt_sb = singles.tile([F, N], mybir.dt.bfloat16)
            for i in range(8):
                nc.scalar.dma_start(xt_sb[:, ts(i, N // 8)], xt[:, ts(i, N // 8)])
            bias_sb = singles.tile([F, 1], mybir.dt.float32)
            nc.scalar.dma_start(bias_sb[:], bias[:])

            # Support S[j, f] = x @ W, in bf16, partitioned as 64 [128, 128]
            # column blocks of s_sb (block jt holds rows jt*128..jt*128+127).
            s_sb = singles.tile([F, N], mybir.dt.bfloat16)
            for jt in range(JT):
                ps = ps_s.tile([F, F], mybir.dt.float32)
                nc.tensor.matmul(
                    ps[:], xt_sb[:, ts(jt, F)], w_sb[:], start=True, stop=True
                )
                nc.vector.tensor_copy(s_sb[:, ts(jt, F)], ps[:])

            # out^T [f, i] = sum_j S[j, f] * A'T[j, i], accumulated over the
            # 64 j-tiles into two PSUM banks (i split 0:512 / 512:1024).
            # The A'T stream is DMAed in TPC-j-tile chunks, alternating
            # between the sync and gpsimd queues to double issue throughput.
            po0 = ps_o.tile([F, 512], mybir.dt.float32)
            po1 = ps_o.tile([F, 512], mybir.dt.float32)
            for ch in range(NCH):
                a_t = atp.tile([F, TPC, RPC], mybir.dt.bfloat16)
                src = at[ch * TPC * F : (ch + 1) * TPC * F, :].rearrange(
                    "(t p) i -> p t i", p=F
                )
                dma_eng = nc.sync if ch % 2 == 0 else nc.gpsimd
                dma_eng.dma_start(a_t[:], src)
                for t in range(TPC):
                    jt = ch * TPC + t
                    st = s_sb[:, ts(jt, F)]
                    first, last = jt == 0, jt == JT - 1
                    nc.tensor.matmul(
                        po0[:], st, a_t[:, t, 0:512], start=first, stop=last
                    )
                    nc.tensor.matmul(
                        po1[:], st, a_t[:, t, 512:1024], start=first, stop=last
                    )

            out_sb = singles.tile([F, RPC], mybir.dt.float32)
            nc.vector.tensor_scalar_add(out_sb[:, 0:512], po0[:], bias_sb[:])
            nc.vector.tensor_scalar_add(out_sb[:, 512:1024], po1[:], bias_sb[:])
            nc.sync.dma_start(out[:], out_sb[:])

    nc.compile()
    return nc


def _get_graph():
    if "nc" not in _graph_cache:
        _graph_cache["nc"] = _build_graph()
    return _graph_cache["nc"]


def _prepare_in_maps(x, adj, edge_w, weight, bias):
    x = np.asarray(x, dtype=np.float32)
    adj = np.asarray(adj).astype(np.int64)
    edge_w = np.asarray(edge_w, dtype=np.float32)
    weight = np.asarray(weight, dtype=np.float32)
    bias = np.asarray(bias, dtype=np.float32)

    rows, cols = adj[0], adj[1]
    deg = 1.0 + np.bincount(rows, weights=edge_w.astype(np.float64), minlength=N)
    dis = (1.0 / np.sqrt(deg + EPS)).astype(np.float32)

    # A'^T[c, r] = dis[r] * w_e * dis[c]; diagonal gets dis[i]^2 (self loop).
    vals = edge_w * dis[rows] * dis[cols]
    at = np.zeros((N, N), dtype=np.float32)
    np.add.at(at, (cols, rows), vals)
    idx = np.arange(N)
    at[idx, idx] += dis * dis
    atb = at.astype(ml_dtypes.bfloat16)

    xtb = np.ascontiguousarray(x.T).astype(ml_dtypes.bfloat16)
    wb = weight.astype(ml_dtypes.bfloat16)
    bias_col = np.ascontiguousarray(bias.reshape(F, 1))

    return [
        {
            "at": np.ascontiguousarray(atb[:, c * RPC : (c + 1) * RPC]),
            "xt": xtb,
            "w": wb,
            "bias": bias_col,
        }
        for c in range(NCORES)
    ]


def _run(in_maps, trace=False, tmpdir=None):
    from concourse.bass_utils import run_bass_kernel_spmd

    nc = _get_graph()
    return run_bass_kernel_spmd(
        nc, in_maps, core_ids=list(range(NCORES)), trace=trace, tmpdir=tmpdir
    )


def _assemble(results):
    return np.ascontiguousarray(
        np.concatenate([results[c]["out"].T for c in range(NCORES)], axis=0)
    ).astype(np.float32)


def kernel(x, adj, edge_w, weight, bias):
    in_maps = _prepare_in_maps(x, adj, edge_w, weight, bias)
    res = _run(in_maps, trace=False)
    return _assemble(res.results)


def kernel_traced(x, adj, edge_w, weight, bias, tmpdir=None):
    """Same as kernel() but profiles the NEFF; returns (output, BassKernelResults)."""
    in_maps = _prepare_in_maps(x, adj, edge_w, weight, bias)
    res = _run(in_maps, trace=True, tmpdir=tmpdir)
    return _assemble(res.results), res


# revision 8
# speedup vs baseline: 1.1035x; 1.0685x over previous
"""GCN layer on 8 TRN2 NeuronCores.

Computation (matches the reference):
    support  = x @ weight                          # [N, F]
    A        = scatter(adj, edge_w) + I            # dense [N, N], duplicate edges sum
    deg      = A.sum(axis=1)
    dis      = 1/sqrt(deg + 1e-10)
    out      = (dis[:,None] * A * dis[None,:]) @ support + bias

Strategy: fold the degree normalization into the dense adjacency on the host
(cheap O(E)/O(N) index work), materialize A'^T = (dis_r * w * dis_c) scattered
at [c, r] in bf16, and row-shard the propagation across 8 cores (1024 output
rows each).  Each core:
  - computes the full support S = x @ W on-device (x^T replicated, bf16),
  - streams its 8192x1024 bf16 A'^T shard from HBM as the matmul moving
    operand, accumulating out^T = S^T @ A'^T in PSUM over 64 k-tiles,
  - adds bias and writes its out^T chunk [128, 1024] f32.
The host transposes/concatenates the 8 chunks into the [8192, 128] output.
"""

import numpy as np
import ml_dtypes

N = 8192
F = 128
NCORES = 8
RPC = N // NCORES  # 1024 rows per core
JT = N // 128  # 64 contraction tiles
EPS = 1e-10

_graph_cache = {}


def _build_graph():
    import concourse.tile as tile
    from concourse import bacc, mybir
    from concourse.bass import ts

    nc = bacc.Bacc("TRN2", target_bir_lowering=False, debug=False, num_devices=NCORES)
    # at is partition-major: at[p, jt, i] = A'^T[jt*128 + p, i] so each SBUF
    # partition line is one long contiguous DRAM read.
    at = nc.declare_dram_parameter("at", [F, JT, RPC], mybir.dt.bfloat16, isOutput=False)
    xt = nc.declare_dram_parameter("xt", [F, N], mybir.dt.bfloat16, isOutput=False)
    w = nc.declare_dram_parameter("w", [F, F], mybir.dt.bfloat16, isOutput=False)
    bias = nc.declare_dram_parameter("bias", [F, 1], mybir.dt.float32, isOutput=False)
    out = nc.declare_dram_parameter("out", [F, RPC], mybir.dt.float32, isOutput=True)

    TPC = 4  # j-tiles per DMA chunk
    NCH = JT // TPC  # 16 chunks
    with tile.TileContext(nc) as tc:
        with (
            tc.tile_pool(name="singles", bufs=1) as singles,
            tc.tile_pool(name="atp", bufs=8) as atp,
            tc.tile_pool(name="ps_s", bufs=4, space="PSUM") as ps_s,
            tc.tile_pool(name="ps_o", bufs=1, space="PSUM") as ps_o,
        ):
            # Weights/x^T on the vector queue so they don't serialize behind
            # the adjacency stream (each dma_start costs ~0.7us of issue time
            # on its sequencer).
            w_sb = singles.tile([F, F], mybir.dt.bfloat16)
            nc.scalar.dma_start(w_sb[:], w[:])
            xt_sb = singles.tile([F, N], mybir.dt.bfloat16)
            for i in range(8):
                nc.scalar.dma_start(xt_sb[:, ts(i, N // 8)], xt[:, ts(i, N // 8)])
            bias_sb = singles.tile([F, 1], mybir.dt.float32)
            nc.scalar.dma_start(bias_sb[:], bias[:])

            # Support S[j, f] = x @ W, in bf16, partitioned as 64 [128, 128]
            # column blocks of s_sb (block jt holds rows jt*128..jt*128+127).
            s_sb = singles.tile([F, N], mybir.dt.bfloat16)
            for jt in range(JT):
                ps = ps_s.tile([F, F], mybir.dt.float32)
                nc.tensor.matmul(
                    ps[:], xt_sb[:, ts(jt, F)], w_sb[:], start=True, stop=True
                )
                nc.vector.tensor_copy(s_sb[:, ts(jt, F)], ps[:])

            # out^T [f, i] = sum_j S[j, f] * A'T[j, i], accumulated over the
            # 64 j-tiles into two PSUM banks (i split 0:512 / 512:1024).
            # The A'T stream is DMAed in TPC-j-tile chunks, alternating
            # between the sync and scalar HWDGE queues to double issue
            # throughput.
            po0 = ps_o.tile([F, 512], mybir.dt.float32)
            po1 = ps_o.tile([F, 512], mybir.dt.float32)
            for ch in range(NCH):
                a_t = atp.tile([F, TPC, RPC], mybir.dt.bfloat16)
                src = at[:, ch * TPC : (ch + 1) * TPC, :]
                nc.sync.dma_start(a_t[:], src)
                for t in range(TPC):
                    jt = ch * TPC + t
                    st = s_sb[:, ts(jt, F)]
                    first, last = jt == 0, jt == JT - 1
                    nc.tensor.matmul(
                        po0[:], st, a_t[:, t, 0:512], start=first, stop=last
                    )
                    nc.tensor.matmul(
                        po1[:], st, a_t[:, t, 512:1024], start=first, stop=last
                    )

            out_sb = singles.tile([F, RPC], mybir.dt.float32)
            nc.vector.tensor_scalar_add(out_sb[:, 0:512], po0[:], bias_sb[:])
            nc.vector.tensor_scalar_add(out_sb[:, 512:1024], po1[:], bias_sb[:])
            nc.sync.dma_start(out[:], out_sb[:])

    nc.compile()
    return nc


def _get_graph():
    if "nc" not in _graph_cache:
        _graph_cache["nc"] = _build_graph()
    return _graph_cache["nc"]


def _prepare_in_maps(x, adj, edge_w, weight, bias):
    x = np.asarray(x, dtype=np.float32)
    adj = np.asarray(adj).astype(np.int64)
    edge_w = np.asarray(edge_w, dtype=np.float32)
    weight = np.asarray(weight, dtype=np.float32)
    bias = np.asarray(bias, dtype=np.float32)

    rows, cols = adj[0], adj[1]
    deg = 1.0 + np.bincount(rows, weights=edge_w.astype(np.float64), minlength=N)
    dis = (1.0 / np.sqrt(deg + EPS)).astype(np.float32)

    # A'^T[c, r] = dis[r] * w_e * dis[c]; diagonal gets dis[i]^2 (self loop).
    vals = edge_w * dis[rows] * dis[cols]
    at = np.zeros((N, N), dtype=np.float32)
    np.add.at(at, (cols, rows), vals)
    idx = np.arange(N)
    at[idx, idx] += dis * dis
    atb = at.astype(ml_dtypes.bfloat16)

    xtb = np.ascontiguousarray(x.T).astype(ml_dtypes.bfloat16)
    wb = weight.astype(ml_dtypes.bfloat16)
    bias_col = np.ascontiguousarray(bias.reshape(F, 1))

    return [
        {
            # [8192, RPC] shard -> partition-major [128, 64, RPC]
            "at": np.ascontiguousarray(
                atb[:, c * RPC : (c + 1) * RPC]
                .reshape(JT, F, RPC)
                .transpose(1, 0, 2)
            ),
            "xt": xtb,
            "w": wb,
            "bias": bias_col,
        }
        for c in range(NCORES)
    ]


def _run(in_maps, trace=False, tmpdir=None):
    from concourse.bass_utils import run_bass_kernel_spmd

    nc = _get_graph()
    return run_bass_kernel_spmd(
        nc, in_maps, core_ids=list(range(NCORES)), trace=trace, tmpdir=tmpdir
    )


def _assemble(results):
    return np.ascontiguousarray(
        np.concatenate([results[c]["out"].T for c in range(NCORES)], axis=0)
    ).astype(np.float32)


def kernel(x, adj, edge_w, weight, bias):
    in_maps = _prepare_in_maps(x, adj, edge_w, weight, bias)
    res = _run(in_maps, trace=False)
    return _assemble(res.results)


def kernel_traced(x, adj, edge_w, weight, bias, tmpdir=None):
    """Same as kernel() but profiles the NEFF; returns (output, BassKernelResults)."""
    in_maps = _prepare_in_maps(x, adj, edge_w, weight, bias)
    res = _run(in_maps, trace=True, tmpdir=tmpdir)
    return _assemble(res.results), res


# revision 9
# speedup vs baseline: 1.2017x; 1.0890x over previous
"""GCN layer on 8 TRN2 NeuronCores.

Computation (matches the reference):
    support  = x @ weight                          # [N, F]
    A        = scatter(adj, edge_w) + I            # dense [N, N], duplicate edges sum
    deg      = A.sum(axis=1)
    dis      = 1/sqrt(deg + 1e-10)
    out      = (dis[:,None] * A * dis[None,:]) @ support + bias

Strategy: fold the degree normalization into the dense adjacency on the host
(cheap O(E)/O(N) index work), materialize A'^T = (dis_r * w * dis_c) scattered
at [c, r] in bf16, and row-shard the propagation across 8 cores (1024 output
rows each).  Per core, using the reassociation
    out^T = W^T @ (x^T @ A'^T) + bias,
the TensorEngine streams the core's 8192x1024 bf16 A'^T shard from HBM as the
moving operand with 128x128 x-tiles stationary, accumulating
P = x^T @ A'^T [128k x 1024i] in PSUM over 64 j-tiles, then applies W^T and
bias.  All heavy DRAM traffic (16 MB/core) is laid out partition-major so
every DMA descriptor line is >=8 KB contiguous.
"""

import numpy as np
import ml_dtypes

N = 8192
F = 128
NCORES = 8
RPC = N // NCORES  # 1024 rows per core
JT = N // 128  # 64 contraction tiles
EPS = 1e-10

_graph_cache = {}


def _build_graph():
    import concourse.tile as tile
    from concourse import bacc, mybir
    from concourse.bass import ts

    nc = bacc.Bacc("TRN2", target_bir_lowering=False, debug=False, num_devices=NCORES)
    # at is partition-major: at[p, jt, i] = A'^T[jt*128 + p, i] so each SBUF
    # partition line is one long contiguous DRAM read.
    at = nc.declare_dram_parameter("at", [F, JT, RPC], mybir.dt.bfloat16, isOutput=False)
    # xp is partition-major x: xp[p, jt, k] = x[jt*128 + p, k]
    xp = nc.declare_dram_parameter("xp", [F, JT, F], mybir.dt.bfloat16, isOutput=False)
    w = nc.declare_dram_parameter("w", [F, F], mybir.dt.bfloat16, isOutput=False)
    bias = nc.declare_dram_parameter("bias", [F, 1], mybir.dt.float32, isOutput=False)
    out = nc.declare_dram_parameter("out", [F, RPC], mybir.dt.float32, isOutput=True)

    TPC = 4  # j-tiles per DMA chunk
    NCH = JT // TPC  # 16 chunks
    with tile.TileContext(nc) as tc:
        with (
            tc.tile_pool(name="singles", bufs=1) as singles,
            tc.tile_pool(name="atp", bufs=8) as atp,
            tc.tile_pool(name="ps_p", bufs=1, space="PSUM") as ps_p,
            tc.tile_pool(name="ps_o", bufs=1, space="PSUM") as ps_o,
        ):
            # x / weight / bias go through the gpsimd SWDGE queue so they do
            # not share HWDGE queues with the adjacency stream.
            x_sb = singles.tile([F, JT, F], mybir.dt.bfloat16)
            nc.gpsimd.dma_start(x_sb[:], xp[:])
            w_sb = singles.tile([F, F], mybir.dt.bfloat16)
            nc.gpsimd.dma_start(w_sb[:], w[:])
            bias_sb = singles.tile([F, 1], mybir.dt.float32)
            nc.gpsimd.dma_start(bias_sb[:], bias[:])

            # P [k, i] = sum_j x[j, k] * A'T[j, i], accumulated over the 64
            # j-tiles into two PSUM banks (i split 0:512 / 512:1024).  The
            # A'T stream is DMAed in TPC-j-tile chunks on the sync HWDGE
            # queues.
            pp0 = ps_p.tile([F, 512], mybir.dt.float32)
            pp1 = ps_p.tile([F, 512], mybir.dt.float32)
            for ch in range(NCH):
                a_t = atp.tile([F, TPC, RPC], mybir.dt.bfloat16)
                nc.sync.dma_start(a_t[:], at[:, ch * TPC : (ch + 1) * TPC, :])
                for t in range(TPC):
                    jt = ch * TPC + t
                    first, last = jt == 0, jt == JT - 1
                    xt_tile = x_sb[:, jt, :]
                    nc.tensor.matmul(
                        pp0[:], xt_tile, a_t[:, t, 0:512], start=first, stop=last
                    )
                    nc.tensor.matmul(
                        pp1[:], xt_tile, a_t[:, t, 512:1024], start=first, stop=last
                    )

            # out^T = W^T @ P + bias
            p_sb = singles.tile([F, RPC], mybir.dt.bfloat16)
            nc.vector.tensor_copy(p_sb[:, 0:512], pp0[:])
            nc.vector.tensor_copy(p_sb[:, 512:1024], pp1[:])
            po0 = ps_o.tile([F, 512], mybir.dt.float32)
            po1 = ps_o.tile([F, 512], mybir.dt.float32)
            nc.tensor.matmul(po0[:], w_sb[:], p_sb[:, 0:512], start=True, stop=True)
            nc.tensor.matmul(po1[:], w_sb[:], p_sb[:, 512:1024], start=True, stop=True)

            out_sb = singles.tile([F, RPC], mybir.dt.float32)
            nc.vector.tensor_scalar_add(out_sb[:, 0:512], po0[:], bias_sb[:])
            nc.vector.tensor_scalar_add(out_sb[:, 512:1024], po1[:], bias_sb[:])
            nc.sync.dma_start(out[:], out_sb[:])

    nc.compile()
    return nc


def _get_graph():
    if "nc" not in _graph_cache:
        _graph_cache["nc"] = _build_graph()
    return _graph_cache["nc"]


def _prepare_in_maps(x, adj, edge_w, weight, bias):
    x = np.asarray(x, dtype=np.float32)
    adj = np.asarray(adj).astype(np.int64)
    edge_w = np.asarray(edge_w, dtype=np.float32)
    weight = np.asarray(weight, dtype=np.float32)
    bias = np.asarray(bias, dtype=np.float32)

    rows, cols = adj[0], adj[1]
    deg = 1.0 + np.bincount(rows, weights=edge_w.astype(np.float64), minlength=N)
    dis = (1.0 / np.sqrt(deg + EPS)).astype(np.float32)

    # A'^T[c, r] = dis[r] * w_e * dis[c]; diagonal gets dis[i]^2 (self loop).
    vals = edge_w * dis[rows] * dis[cols]
    at = np.zeros((N, N), dtype=np.float32)
    np.add.at(at, (cols, rows), vals)
    idx = np.arange(N)
    at[idx, idx] += dis * dis
    atb = at.astype(ml_dtypes.bfloat16)

    # partition-major x: [8192, 128] -> [128, 64, 128]
    xpb = np.ascontiguousarray(
        x.astype(ml_dtypes.bfloat16).reshape(JT, F, F).transpose(1, 0, 2)
    )
    wb = weight.astype(ml_dtypes.bfloat16)
    bias_col = np.ascontiguousarray(bias.reshape(F, 1))

    return [
        {
            # [8192, RPC] shard -> partition-major [128, 64, RPC]
            "at": np.ascontiguousarray(
                atb[:, c * RPC : (c + 1) * RPC]
                .reshape(JT, F, RPC)
                .transpose(1, 0, 2)
            ),
            "xp": xpb,
            "w": wb,
            "bias": bias_col,
        }
        for c in range(NCORES)
    ]


def _run(in_maps, trace=False, tmpdir=None):
    from concourse.bass_utils import run_bass_kernel_spmd

    nc = _get_graph()
    return run_bass_kernel_spmd(
        nc, in_maps, core_ids=list(range(NCORES)), trace=trace, tmpdir=tmpdir
    )


def _assemble(results):
    return np.ascontiguousarray(
        np.concatenate([results[c]["out"].T for c in range(NCORES)], axis=0)
    ).astype(np.float32)


def kernel(x, adj, edge_w, weight, bias):
    in_maps = _prepare_in_maps(x, adj, edge_w, weight, bias)
    res = _run(in_maps, trace=False)
    return _assemble(res.results)


def kernel_traced(x, adj, edge_w, weight, bias, tmpdir=None):
    """Same as kernel() but profiles the NEFF; returns (output, BassKernelResults)."""
    in_maps = _prepare_in_maps(x, adj, edge_w, weight, bias)
    res = _run(in_maps, trace=True, tmpdir=tmpdir)
    return _assemble(res.results), res


# revision 11
# speedup vs baseline: 1.2119x; 1.0085x over previous
"""GCN layer on 8 TRN2 NeuronCores.

Computation (matches the reference):
    support  = x @ weight                          # [N, F]
    A        = scatter(adj, edge_w) + I            # dense [N, N], duplicate edges sum
    deg      = A.sum(axis=1)
    dis      = 1/sqrt(deg + 1e-10)
    out      = (dis[:,None] * A * dis[None,:]) @ support + bias

Strategy: fold the degree normalization into the dense adjacency on the host
(cheap O(E)/O(N) index work), materialize A'^T = (dis_r * w * dis_c) scattered
at [c, r] in bf16, and row-shard the propagation across 8 cores (1024 output
rows each).  Per core, using the reassociation
    out^T = W^T @ (x^T @ A'^T) + bias,
the TensorEngine streams the core's 8192x1024 bf16 A'^T shard from HBM as the
moving operand with 128x128 x-tiles stationary, accumulating
P = x^T @ A'^T [128k x 1024i] in PSUM over 64 j-tiles, then applies W^T and
bias.  All heavy DRAM traffic (16 MB/core) is laid out partition-major so
every DMA descriptor line is >=8 KB contiguous.
"""

import numpy as np
import ml_dtypes

N = 8192
F = 128
NCORES = 8
RPC = N // NCORES  # 1024 rows per core
JT = N // 128  # 64 contraction tiles
EPS = 1e-10

_graph_cache = {}


def _build_graph():
    import concourse.tile as tile
    from concourse import bacc, mybir
    from concourse.bass import ts

    nc = bacc.Bacc("TRN2", target_bir_lowering=False, debug=False, num_devices=NCORES)
    # at is partition-major: at[p, jt, i] = A'^T[jt*128 + p, i] so each SBUF
    # partition line is one long contiguous DRAM read.
    at = nc.declare_dram_parameter("at", [F, JT, RPC], mybir.dt.bfloat16, isOutput=False)
    # xp is partition-major x: xp[p, jt, k] = x[jt*128 + p, k]
    xp = nc.declare_dram_parameter("xp", [F, JT, F], mybir.dt.bfloat16, isOutput=False)
    w = nc.declare_dram_parameter("w", [F, F], mybir.dt.bfloat16, isOutput=False)
    bias = nc.declare_dram_parameter("bias", [F, 1], mybir.dt.float32, isOutput=False)
    out = nc.declare_dram_parameter("out", [F, RPC], mybir.dt.float32, isOutput=True)

    TPC = 4  # j-tiles per DMA chunk
    NCH = JT // TPC  # 16 chunks
    with tile.TileContext(nc) as tc:
        with (
            tc.tile_pool(name="singles", bufs=1) as singles,
            tc.tile_pool(name="atp", bufs=8) as atp,
            tc.tile_pool(name="ps_p", bufs=1, space="PSUM") as ps_p,
            tc.tile_pool(name="ps_o", bufs=1, space="PSUM") as ps_o,
        ):
            # x leads the sync HWDGE queue, split into 4 separately-tracked
            # tiles so the first j-tile matmuls only depend on the first
            # quarter.  weight/bias ride the gpsimd SWDGE queue (only needed
            # for the epilogue).
            XCH = 4
            x_sbs = []
            for i in range(XCH):
                x_sb_i = singles.tile([F, JT // XCH, F], mybir.dt.bfloat16, tag=f"x{i}")
                nc.sync.dma_start(
                    x_sb_i[:], xp[:, i * (JT // XCH) : (i + 1) * (JT // XCH), :]
                )
                x_sbs.append(x_sb_i)
            w_sb = singles.tile([F, F], mybir.dt.bfloat16)
            nc.gpsimd.dma_start(w_sb[:], w[:])
            bias_sb = singles.tile([F, 1], mybir.dt.float32)
            nc.gpsimd.dma_start(bias_sb[:], bias[:])

            # P [k, i] = sum_j x[j, k] * A'T[j, i], accumulated over the 64
            # j-tiles into two PSUM banks (i split 0:512 / 512:1024).  The
            # A'T stream is DMAed in TPC-j-tile chunks on the sync HWDGE
            # queues.
            pp0 = ps_p.tile([F, 512], mybir.dt.float32)
            pp1 = ps_p.tile([F, 512], mybir.dt.float32)
            for ch in range(NCH):
                a_t = atp.tile([F, TPC, RPC], mybir.dt.bfloat16)
                nc.sync.dma_start(a_t[:], at[:, ch * TPC : (ch + 1) * TPC, :])
                for t in range(TPC):
                    jt = ch * TPC + t
                    first, last = jt == 0, jt == JT - 1
                    xt_tile = x_sbs[jt // (JT // XCH)][:, jt % (JT // XCH), :]
                    nc.tensor.matmul(
                        pp0[:], xt_tile, a_t[:, t, 0:512], start=first, stop=last
                    )
                    nc.tensor.matmul(
                        pp1[:], xt_tile, a_t[:, t, 512:1024], start=first, stop=last
                    )

            # out^T = W^T @ P + bias
            p_sb = singles.tile([F, RPC], mybir.dt.bfloat16)
            nc.vector.tensor_copy(p_sb[:, 0:512], pp0[:])
            nc.vector.tensor_copy(p_sb[:, 512:1024], pp1[:])
            po0 = ps_o.tile([F, 512], mybir.dt.float32)
            po1 = ps_o.tile([F, 512], mybir.dt.float32)
            nc.tensor.matmul(po0[:], w_sb[:], p_sb[:, 0:512], start=True, stop=True)
            nc.tensor.matmul(po1[:], w_sb[:], p_sb[:, 512:1024], start=True, stop=True)

            out_sb = singles.tile([F, RPC], mybir.dt.float32)
            nc.vector.tensor_scalar_add(out_sb[:, 0:512], po0[:], bias_sb[:])
            nc.vector.tensor_scalar_add(out_sb[:, 512:1024], po1[:], bias_sb[:])
            nc.sync.dma_start(out[:], out_sb[:])

    nc.compile()
    return nc


def _get_graph():
    if "nc" not in _graph_cache:
        _graph_cache["nc"] = _build_graph()
    return _graph_cache["nc"]


def _prepare_in_maps(x, adj, edge_w, weight, bias):
    x = np.asarray(x, dtype=np.float32)
    adj = np.asarray(adj).astype(np.int64)
    edge_w = np.asarray(edge_w, dtype=np.float32)
    weight = np.asarray(weight, dtype=np.float32)
    bias = np.asarray(bias, dtype=np.float32)

    rows, cols = adj[0], adj[1]
    deg = 1.0 + np.bincount(rows, weights=edge_w.astype(np.float64), minlength=N)
    dis = (1.0 / np.sqrt(deg + EPS)).astype(np.float32)

    # A'^T[c, r] = dis[r] * w_e * dis[c]; diagonal gets dis[i]^2 (self loop).
    vals = edge_w * dis[rows] * dis[cols]
    at = np.zeros((N, N), dtype=np.float32)
    np.add.at(at, (cols, rows), vals)
    idx = np.arange(N)
    at[idx, idx] += dis * dis
    atb = at.astype(ml_dtypes.bfloat16)

    # partition-major x: [8192, 128] -> [128, 64, 128]
    xpb = np.ascontiguousarray(
        x.astype(ml_dtypes.bfloat16).reshape(JT, F, F).transpose(1, 0, 2)
    )
    wb = weight.astype(ml_dtypes.bfloat16)
    bias_col = np.ascontiguousarray(bias.reshape(F, 1))

    return [
        {
            # [8192, RPC] shard -> partition-major [128, 64, RPC]
            "at": np.ascontiguousarray(
                atb[:, c * RPC : (c + 1) * RPC]
                .reshape(JT, F, RPC)
                .transpose(1, 0, 2)
            ),
            "xp": xpb,
            "w": wb,
            "bias": bias_col,
        }
        for c in range(NCORES)
    ]


def _run(in_maps, trace=False, tmpdir=None):
    from concourse.bass_utils import run_bass_kernel_spmd

    nc = _get_graph()
    return run_bass_kernel_spmd(
        nc, in_maps, core_ids=list(range(NCORES)), trace=trace, tmpdir=tmpdir
    )


def _assemble(results):
    return np.ascontiguousarray(
        np.concatenate([results[c]["out"].T for c in range(NCORES)], axis=0)
    ).astype(np.float32)


def kernel(x, adj, edge_w, weight, bias):
    in_maps = _prepare_in_maps(x, adj, edge_w, weight, bias)
    res = _run(in_maps, trace=False)
    return _assemble(res.results)


def kernel_traced(x, adj, edge_w, weight, bias, tmpdir=None):
    """Same as kernel() but profiles the NEFF; returns (output, BassKernelResults)."""
    in_maps = _prepare_in_maps(x, adj, edge_w, weight, bias)
    res = _run(in_maps, trace=True, tmpdir=tmpdir)
    return _assemble(res.results), res


# revision 15
# speedup vs baseline: 1.2323x; 1.0168x over previous
"""GCN layer on 8 TRN2 NeuronCores.

Computation (matches the reference):
    support  = x @ weight                          # [N, F]
    A        = scatter(adj, edge_w) + I            # dense [N, N], duplicate edges sum
    deg      = A.sum(axis=1)
    dis      = 1/sqrt(deg + 1e-10)
    out      = (dis[:,None] * A * dis[None,:]) @ support + bias

Strategy: fold the degree normalization into the dense adjacency on the host
(cheap O(E)/O(N) index work), materialize A'^T = (dis_r * w * dis_c) scattered
at [c, r] in bf16, and row-shard the propagation across 8 cores (1024 output
rows each).  Per core, using the reassociation
    out^T = W^T @ (x^T @ A'^T) + bias,
the TensorEngine streams the core's 8192x1024 bf16 A'^T shard from HBM as the
moving operand with 128x128 x-tiles stationary, accumulating
P = x^T @ A'^T [128k x 1024i] in PSUM over 64 j-tiles, then applies W^T and
bias.  All heavy DRAM traffic (16 MB/core) is laid out partition-major so
every DMA descriptor line is >=8 KB contiguous.
"""

import numpy as np
import ml_dtypes

N = 8192
F = 128
NCORES = 8
RPC = N // NCORES  # 1024 rows per core
JT = N // 128  # 64 contraction tiles
EPS = 1e-10

_graph_cache = {}


def _build_graph():
    import concourse.tile as tile
    from concourse import bacc, mybir
    from concourse.bass import ts

    nc = bacc.Bacc("TRN2", target_bir_lowering=False, debug=False, num_devices=NCORES)
    # at is partition-major: at[p, jt, i] = A'^T[jt*128 + p, i] so each SBUF
    # partition line is one long contiguous DRAM read.
    at = nc.declare_dram_parameter("at", [F, JT, RPC], mybir.dt.bfloat16, isOutput=False)
    # xp is partition-major x: xp[p, jt, k] = x[jt*128 + p, k]
    xp = nc.declare_dram_parameter("xp", [F, JT, F], mybir.dt.bfloat16, isOutput=False)
    w = nc.declare_dram_parameter("w", [F, F], mybir.dt.bfloat16, isOutput=False)
    bias = nc.declare_dram_parameter("bias", [F, 1], mybir.dt.float32, isOutput=False)
    out = nc.declare_dram_parameter("out", [F, RPC], mybir.dt.bfloat16, isOutput=True)

    TPC = 4  # j-tiles per DMA chunk
    NCH = JT // TPC  # 16 chunks
    with tile.TileContext(nc) as tc:
        with (
            tc.tile_pool(name="singles", bufs=1) as singles,
            tc.tile_pool(name="atp", bufs=8) as atp,
            tc.tile_pool(name="ps_p", bufs=1, space="PSUM") as ps_p,
            tc.tile_pool(name="ps_o", bufs=1, space="PSUM") as ps_o,
        ):
            # x is split into 4 separately-tracked tiles interleaved into the
            # adjacency stream just-in-time (quarter q is only needed from
            # at-chunk 4q on).  weight/bias ride the gpsimd SWDGE queue (only
            # needed for the epilogue).
            XCH = 4
            XQ = JT // XCH
            x_sbs = [
                singles.tile(
                    [F, XQ, F], mybir.dt.bfloat16, name=f"x{i}", tag=f"x{i}"
                )
                for i in range(XCH)
            ]
            w_sb = singles.tile([F, F], mybir.dt.bfloat16)
            nc.gpsimd.dma_start(w_sb[:], w[:])
            bias_sb = singles.tile([F, 1], mybir.dt.float32)
            nc.gpsimd.dma_start(bias_sb[:], bias[:])

            def load_x(i):
                nc.sync.dma_start(x_sbs[i][:], xp[:, i * XQ : (i + 1) * XQ, :])

            # P [k, i] = sum_j x[j, k] * A'T[j, i], accumulated over the 64
            # j-tiles into two PSUM banks (i split 0:512 / 512:1024).  The
            # A'T stream is DMAed in TPC-j-tile chunks on the sync HWDGE
            # queues; x quarter i+1 is injected after at-chunk 4i so the
            # stream stays just-in-time.
            load_x(0)
            pp0 = ps_p.tile([F, 512], mybir.dt.float32)
            pp1 = ps_p.tile([F, 512], mybir.dt.float32)
            for ch in range(NCH):
                a_t = atp.tile([F, TPC, RPC], mybir.dt.bfloat16)
                nc.sync.dma_start(a_t[:], at[:, ch * TPC : (ch + 1) * TPC, :])
                if ch % XCH == 0 and ch // XCH + 1 < XCH:
                    load_x(ch // XCH + 1)
                for t in range(TPC):
                    jt = ch * TPC + t
                    first, last = jt == 0, jt == JT - 1
                    xt_tile = x_sbs[jt // XQ][:, jt % XQ, :]
                    nc.tensor.matmul(
                        pp0[:], xt_tile, a_t[:, t, 0:512], start=first, stop=last
                    )
                    nc.tensor.matmul(
                        pp1[:], xt_tile, a_t[:, t, 512:1024], start=first, stop=last
                    )

            # out^T = W^T @ P + bias
            p_sb = singles.tile([F, RPC], mybir.dt.bfloat16)
            nc.vector.tensor_copy(p_sb[:, 0:512], pp0[:])
            nc.vector.tensor_copy(p_sb[:, 512:1024], pp1[:])
            po0 = ps_o.tile([F, 512], mybir.dt.float32)
            po1 = ps_o.tile([F, 512], mybir.dt.float32)
            nc.tensor.matmul(po0[:], w_sb[:], p_sb[:, 0:512], start=True, stop=True)
            nc.tensor.matmul(po1[:], w_sb[:], p_sb[:, 512:1024], start=True, stop=True)

            out_sb = singles.tile([F, RPC], mybir.dt.bfloat16)
            nc.vector.tensor_scalar_add(out_sb[:, 0:512], po0[:], bias_sb[:])
            nc.vector.tensor_scalar_add(out_sb[:, 512:1024], po1[:], bias_sb[:])
            nc.sync.dma_start(out[:], out_sb[:])

    nc.compile()
    return nc


def _get_graph():
    if "nc" not in _graph_cache:
        _graph_cache["nc"] = _build_graph()
    return _graph_cache["nc"]


def _prepare_in_maps(x, adj, edge_w, weight, bias):
    x = np.asarray(x, dtype=np.float32)
    adj = np.asarray(adj).astype(np.int64)
    edge_w = np.asarray(edge_w, dtype=np.float32)
    weight = np.asarray(weight, dtype=np.float32)
    bias = np.asarray(bias, dtype=np.float32)

    rows, cols = adj[0], adj[1]
    deg = 1.0 + np.bincount(rows, weights=edge_w.astype(np.float64), minlength=N)
    dis = (1.0 / np.sqrt(deg + EPS)).astype(np.float32)

    # A'^T[c, r] = dis[r] * w_e * dis[c]; diagonal gets dis[i]^2 (self loop).
    vals = edge_w * dis[rows] * dis[cols]
    at = np.zeros((N, N), dtype=np.float32)
    np.add.at(at, (cols, rows), vals)
    idx = np.arange(N)
    at[idx, idx] += dis * dis
    atb = at.astype(ml_dtypes.bfloat16)

    # partition-major x: [8192, 128] -> [128, 64, 128]
    xpb = np.ascontiguousarray(
        x.astype(ml_dtypes.bfloat16).reshape(JT, F, F).transpose(1, 0, 2)
    )
    wb = weight.astype(ml_dtypes.bfloat16)
    bias_col = np.ascontiguousarray(bias.reshape(F, 1))

    return [
        {
            # [8192, RPC] shard -> partition-major [128, 64, RPC]
            "at": np.ascontiguousarray(
                atb[:, c * RPC : (c + 1) * RPC]
                .reshape(JT, F, RPC)
                .transpose(1, 0, 2)
            ),
            "xp": xpb,
            "w": wb,
            "bias": bias_col,
        }
        for c in range(NCORES)
    ]


def _run(in_maps, trace=False, tmpdir=None):
    from concourse.bass_utils import run_bass_kernel_spmd

    nc = _get_graph()
    return run_bass_kernel_spmd(
        nc, in_maps, core_ids=list(range(NCORES)), trace=trace, tmpdir=tmpdir
    )


def _assemble(results):
    return np.ascontiguousarray(
        np.concatenate([results[c]["out"].T for c in range(NCORES)], axis=0)
    ).astype(np.float32)


def kernel(x, adj, edge_w, weight, bias):
    in_maps = _prepare_in_maps(x, adj, edge_w, weight, bias)
    res = _run(in_maps, trace=False)
    return _assemble(res.results)


def kernel_traced(x, adj, edge_w, weight, bias, tmpdir=None):
    """Same as kernel() but profiles the NEFF; returns (output, BassKernelResults)."""
    in_maps = _prepare_in_maps(x, adj, edge_w, weight, bias)
    res = _run(in_maps, trace=True, tmpdir=tmpdir)
    return _assemble(res.results), res
